# revision 2
# baseline (speedup 1.0000x reference)
"""Cumulative (causal) LayerNorm Trainium2 Bass kernel — fp16-IO pipeline.

Problem: inputs [B=8, K=8000, H=512] f32, gamma/beta [1, 512].
At step k, normalize frame k by mean/var computed over the prefix
inputs[:, :k+1, :], then scale/shift by gamma/beta.

Sharding: data-parallel over batch B across the 8 NeuronCores (one sample
per core), gamma/beta/constants replicated. No cross-core communication.

Key measured facts on this fabric (see perf.py experiments):
  - All DMA queues time-share one SDMA engine pool; splitting a direction
    across queues SLOWS it down. One SWDGE (gpsimd) queue carrying both
    loads and stores interleaved is fastest (mixed read/write stream hides
    HBM read latency).
  - SWDGE beats HWDGE for bulk transfers here (~86µs vs ~117µs for 16MB
    loads); 8KiB-per-partition runs are the sweet spot.
  - The 2e-2 rel-err gate admits fp16 I/O: host casts x/gamma to fp16 and
    the output back to f32 (max rel err 2.9e-3, verified vs reference).
    HBM traffic halves: 8.2MB in + 8.2MB out per core.

Per-core layout (segmented): K = NSEG * P * FS frames; frame
    k = s*(P*FS) + p*FS + f      (s = segment, p = partition, f = tile-in-seg)
Engine split per segment:
  gpsimd: ONE SWDGE queue: load chunk(s) of segment s+1 emitted BEFORE the
          stores of segment s so a store's sem-wait never starves the queue.
  ACT:    Square+accum per frame -> per-frame sumsq; Sqrt for invstd.
  DVE:    per-frame sums via tensor_scalar+accum (4x mode on fp16);
          tensor_tensor_scan along each partition's FS frames; cross-
          partition carry adds; stats smalls; fused apply (x*isd + nmb,
          one 4x tensor_scalar); gamma multiply (2x tensor_tensor).
  PE:     strictly-upper-triangular ustrict @ seg-totals (exclusive cross-
          partition carry) + row124-ones @ previous segment's final cum
          column (scalar carry broadcast), accumulated in PSUM.
"""

import numpy as np

import concourse.bass as bass
import concourse.tile as tile
from concourse import bacc, mybir
from concourse import bass_utils

B, K, H = 8, 8000, 512
P = 125           # partitions
NSEG = 8          # segments per sample
FS = 8            # k-tiles per segment  (K = NSEG * P * FS)
F = NSEG * FS     # 64 k-tiles total
IN_CHUNK = 8      # k-tiles per load DMA  (fp16: 8KiB/partition runs)
ST_CHUNK = 8      # k-tiles per store DMA (fp16: 8KiB/partition runs)
EPS = 1e-8
N_CORES = 8

FP32 = mybir.dt.float32
FP16 = mybir.dt.float16


def _global_k(p, t, fs=FS):
    s, f = t // fs, t % fs
    return s * (P * fs) + p * fs + f


def _make_consts(nseg: int = NSEG):
    # invc2[:, 0:F] = -1/count, invc2[:, F:2F] = +1/count, count = (k+1)*H
    pp, tt = np.meshgrid(np.arange(P), np.arange(F), indexing="ij")
    k = _global_k(pp, tt, F // nseg).astype(np.float64)
    inv_cnt = 1.0 / ((k + 1) * H)
    invc2 = np.concatenate([-inv_cnt, inv_cnt], axis=1).astype(np.float32)
    ustrict = np.triu(np.ones((P, P), dtype=np.float32), k=1)
    row124 = np.zeros((P, P), dtype=np.float32)
    row124[P - 1, :] = 1.0
    return invc2, ustrict, row124


def _build_kernel(with_beta: bool, reps: int = 1,
                  gamma_eng: str = "vector", ablate: str = "full",
                  sum_eng: str = "reduce", sq_psum: bool = True,
                  prefetch: int = 2, carry_direct: bool = True) -> bass.Bass:
    # ablate: full | io | compute | dve_sum | act_sq | dve_apply | dve_gamma
    #         | scanstats
    # sum_eng: ts (DVE tensor_scalar+accum) | ttr (DVE ttr vs ones) |
    #          gp (gpsimd batched reduce) | reduce (DVE batched reduce)
    ab = ablate
    do_load = ab in ("full", "io")
    do_store = ab in ("full", "io")
    do_sum = ab in ("full", "compute", "dve_sum")
    do_sq = ab in ("full", "compute", "act_sq")
    do_scan = ab in ("full", "compute", "scanstats")
    do_apply = ab in ("full", "compute", "dve_apply")
    do_gamma = ab in ("full", "compute", "dve_gamma")
    nc = bacc.Bacc("TRN2", target_bir_lowering=False, debug=False,
                   num_devices=N_CORES)
    x_d = nc.dram_tensor("x", [K, H], FP16, kind="ExternalInput").ap()
    gamma_d = nc.dram_tensor("gamma", [1, H], FP16, kind="ExternalInput").ap()
    beta_d = nc.dram_tensor("beta", [1, H], FP16, kind="ExternalInput").ap()
    invc2_d = nc.dram_tensor("invc2", [P, 2 * F], FP32, kind="ExternalInput").ap()
    ustrict_d = nc.dram_tensor("ustrict", [P, P], FP32, kind="ExternalInput").ap()
    row124_d = nc.dram_tensor("row124", [P, P], FP32, kind="ExternalInput").ap()
    out_d = nc.dram_tensor("out", [K, H], FP16, kind="ExternalOutput").ap()

    x_v = x_d.rearrange("(s p f) h -> s p f h", p=P, f=FS)
    out_v = out_d.rearrange("(s p f) h -> s p f h", p=P, f=FS)

    with tile.TileContext(nc) as tc:
        with (
            tc.tile_pool(name="xbuf", bufs=1) as xpool,
            tc.tile_pool(name="small", bufs=1) as small,
            tc.tile_pool(name="psum", bufs=2, space="PSUM") as psum,
        ):
            X = xpool.tile([P, F, H], FP16)   # X[:, t, :], t = s*FS + f

            G = small.tile([P, H], FP16)
            nc.sync.dma_start(G[:, :], gamma_d.to_broadcast((P, H)))
            Bt = None
            if with_beta:
                Bt = small.tile([P, H], FP16, tag="beta")
                nc.sync.dma_start(Bt[:, :], beta_d.to_broadcast((P, H)))
            invc2 = small.tile([P, 2 * F], FP32, tag="invc2")
            nc.sync.dma_start(invc2[:, :], invc2_d)
            ustrict = small.tile([P, P], FP32, tag="ustrict")
            nc.sync.dma_start(ustrict[:, :], ustrict_d)
            row124 = small.tile([P, P], FP32, tag="row124")
            nc.sync.dma_start(row124[:, :], row124_d)
            eps_t = small.tile([P, 1], FP32, tag="eps")
            nc.vector.memset(eps_t[:, :], EPS)
            # zb produced on ACT so the Square+accum's single encodable sync
            # wait can go to the load DMA sem (same trick as the baseline).
            zb = small.tile([P, 1], FP32, tag="zb")
            nc.scalar.memzero(zb[:, :])

            S = small.tile([P, 2 * F], FP32, tag="S")    # sums | sumsqs
            C = small.tile([P, 2 * F], FP32, tag="C")    # global cums
            M = small.tile([P, 2 * F], FP32, tag="M")    # [-mean | E[x^2]]
            Msq = small.tile([P, F], FP32, tag="Msq")
            V = small.tile([P, F], FP32, tag="V")
            ISD = small.tile([P, F], FP32, tag="ISD")
            NMB = small.tile([P, F], FP32, tag="NMB")
            carryS = small.tile([P, 2 * NSEG], FP32, tag="carryS")
            scratch = (psum if sq_psum else small).tile(
                [P, H], FP32, tag="scratch", name="scratch")
            scratch16 = small.tile([P, H], FP16, tag="scratch16")
            ones16 = None
            if sum_eng == "ttr":
                ones16 = small.tile([P, H], FP16, tag="ones16", name="ones16")
                nc.vector.memset(ones16[:, :], 1.0)

            carryP = psum.tile([P, 2], FP32)
            pe_touch = psum.tile([1, 1], FP32, tag="pe_touch")

            Cr = C[:, :].rearrange("p (a b) -> p a b", b=F)
            Mr = M[:, :].rearrange("p (a b) -> p a b", b=F)
            Ir = invc2[:, :].rearrange("p (a b) -> p a b", b=F)

            # absorb the ustrict/row124 DMA wait on PE once
            nc.tensor.matmul(pe_touch[0:1, 0:1], row124[0:1, 0:1],
                             ustrict[0:1, 0:1], start=True, stop=True)

            def emit_loads(s):
                for c in range(max(1, FS // IN_CHUNK)):
                    f0 = s * FS + c * IN_CHUNK
                    nc.gpsimd.dma_start(
                        X[:, f0:f0 + IN_CHUNK, :],
                        x_v[s, :, c * IN_CHUNK:(c + 1) * IN_CHUNK, :])

            if not do_load:
                for s in range(NSEG):
                    emit_loads(s)
            if not (do_sum and do_sq):
                nc.vector.memset(S[:, :], 0.5)
            if not do_scan:
                nc.vector.memset(ISD[:, :], 1.0)
                nc.vector.memset(NMB[:, :], 0.0)

            def rep_body():
              if do_load:
                  for s in range(min(prefetch, NSEG)):
                      emit_loads(s)
              for s in range(NSEG):
                t0 = s * FS
                # prefetch later segments' loads ahead of this segment's
                # stores in the single gpsimd queue (no starvation)
                if do_load and s + prefetch < NSEG:
                    emit_loads(s + prefetch)

                # ---- per-frame stats ------------------------------------
                if do_sum and sum_eng == "reduce":
                    # batched per-frame sums: one DVE op per segment
                    nc.vector.reduce_sum(S[:, t0:t0 + FS], X[:, t0:t0 + FS, :],
                                         axis=mybir.AxisListType.X)
                for f in range(t0, t0 + FS):
                    if do_sq:
                        # sumsq on ACT: Square with accumulate
                        nc.scalar.activation(
                            out=scratch[:, :], in_=X[:, f, :],
                            func=mybir.ActivationFunctionType.Square,
                            bias=zb[:, :], scale=1.0,
                            accum_out=S[:, F + f:F + f + 1],
                        )
                    if do_sum and sum_eng == "ts":
                        # sum on DVE: x*1+0 with accumulate
                        nc.vector.tensor_scalar(
                            out=scratch16[:, :], in0=X[:, f, :],
                            scalar1=1.0, scalar2=0.0,
                            op0=mybir.AluOpType.mult,
                            op1=mybir.AluOpType.add,
                            accum_out=S[:, f:f + 1],
                        )
                    elif do_sum and sum_eng == "ttr":
                        # sum on DVE: (x * ones) reduce-add (TT-class)
                        nc.vector.tensor_tensor_reduce(
                            out=scratch16[:, :], in0=X[:, f, :],
                            in1=ones16[:, :], scale=1.0, scalar=0.0,
                            op0=mybir.AluOpType.mult,
                            op1=mybir.AluOpType.add,
                            accum_out=S[:, f:f + 1],
                        )

                if do_scan:
                    # ---- causal scan for this segment --------------------
                    nc.vector.tensor_tensor_scan(
                        out=C[:, t0:t0 + FS], data0=S[:, t0:t0 + FS],
                        data1=S[:, t0:t0 + FS], initial=0.0,
                        op0=mybir.AluOpType.add, op1=mybir.AluOpType.bypass)
                    nc.vector.tensor_tensor_scan(
                        out=C[:, F + t0:F + t0 + FS],
                        data0=S[:, F + t0:F + t0 + FS],
                        data1=S[:, F + t0:F + t0 + FS], initial=0.0,
                        op0=mybir.AluOpType.add, op1=mybir.AluOpType.bypass)
                    totals = Cr[:, :, t0 + FS - 1]          # [P, 2] strided
                    nc.tensor.matmul(carryP[:, 0:2], ustrict[:, :], totals,
                                     start=True, stop=(s == 0))
                    if s > 0:
                        prevfinal = Cr[:, :, t0 - 1]        # already global
                        nc.tensor.matmul(carryP[:, 0:2], row124[:, :],
                                         prevfinal, start=False, stop=True)
                    if carry_direct:
                        # DVE reads the carry straight from PSUM: no ACT
                        # copy round-trip on the scan critical path
                        cS = carryP
                    else:
                        cS = carryS[:, 2 * s:2 * s + 2]
                        nc.scalar.copy(cS[:, :], carryP[:, :])
                    nc.vector.tensor_scalar_add(C[:, t0:t0 + FS],
                                                C[:, t0:t0 + FS], cS[:, 0:1])
                    nc.vector.tensor_scalar_add(C[:, F + t0:F + t0 + FS],
                                                C[:, F + t0:F + t0 + FS],
                                                cS[:, 1:2])

                    # ---- stats for this segment --------------------------
                    nc.vector.tensor_mul(Mr[:, :, t0:t0 + FS],
                                         Cr[:, :, t0:t0 + FS],
                                         Ir[:, :, t0:t0 + FS])
                    nc.vector.tensor_mul(Msq[:, t0:t0 + FS], M[:, t0:t0 + FS],
                                         M[:, t0:t0 + FS])          # mean^2
                    nc.vector.tensor_sub(V[:, t0:t0 + FS],
                                         M[:, F + t0:F + t0 + FS],
                                         Msq[:, t0:t0 + FS])        # var
                    nc.scalar.activation(out=V[:, t0:t0 + FS],
                                         in_=V[:, t0:t0 + FS],
                                         func=mybir.ActivationFunctionType.Sqrt,
                                         bias=eps_t[:, :], scale=1.0)
                    nc.vector.reciprocal(ISD[:, t0:t0 + FS], V[:, t0:t0 + FS])
                    nc.vector.tensor_mul(NMB[:, t0:t0 + FS], M[:, t0:t0 + FS],
                                         ISD[:, t0:t0 + FS])    # -mean*invstd

                # ---- apply + store for this segment ----------------------
                for c in range(max(1, FS // ST_CHUNK)):
                    f0 = t0 + c * ST_CHUNK
                    for f in range(f0, f0 + ST_CHUNK):
                        if do_apply:
                            # x <- x*invstd + (-mean*invstd): fused DVE (4x)
                            nc.vector.tensor_scalar(
                                out=X[:, f, :], in0=X[:, f, :],
                                scalar1=ISD[:, f:f + 1],
                                scalar2=NMB[:, f:f + 1],
                                op0=mybir.AluOpType.mult,
                                op1=mybir.AluOpType.add)
                        if do_gamma:
                            geng = {"vector": nc.vector, "gpsimd": nc.gpsimd,
                                    "split": (nc.vector if f % 2 == 0
                                              else nc.gpsimd)}[gamma_eng]
                            geng.tensor_mul(X[:, f, :], X[:, f, :], G[:, :])
                            if Bt is not None:
                                geng.tensor_add(X[:, f, :], X[:, f, :],
                                                Bt[:, :])
                    if do_store:
                        nc.gpsimd.dma_start(
                            out_v[s, :, c * ST_CHUNK:(c + 1) * ST_CHUNK, :],
                            X[:, f0:f0 + ST_CHUNK, :])

            if reps == 1:
                rep_body()
            else:
                # hardware loop (all-engine barrier between iterations);
                # used by perf.py for single-shot-span measurement
                with tc.For_i(0, reps, 1):
                    rep_body()

    nc.finalize()
    return nc


_NC_CACHE: dict = {}


def kernel(**inputs: np.ndarray) -> np.ndarray:
    x = np.asarray(inputs["inputs"])
    gamma = np.asarray(inputs["gamma"])
    beta = np.asarray(inputs["beta"])
    assert x.shape == (B, K, H), x.shape

    x16 = np.ascontiguousarray(x.astype(np.float16))
    gamma16 = np.ascontiguousarray(gamma.astype(np.float16)).reshape(1, H)
    beta16 = np.ascontiguousarray(beta.astype(np.float16)).reshape(1, H)

    with_beta = bool(np.any(beta16 != 0))
    key = (with_beta, 1)
    if key not in _NC_CACHE:
        _NC_CACHE[key] = _build_kernel(with_beta, reps=1)
    nc = _NC_CACHE[key]

    invc2, ustrict, row124 = _make_consts()
    in_maps = [
        {
            "x": np.ascontiguousarray(x16[b]),
            "gamma": gamma16,
            "beta": beta16,
            "invc2": invc2,
            "ustrict": ustrict,
            "row124": row124,
        }
        for b in range(B)
    ]
    res = bass_utils.run_bass_kernel_spmd(nc, in_maps, core_ids=list(range(N_CORES)))
    out = np.stack([res.results[b]["out"] for b in range(B)], axis=0)
    return out.astype(np.float32)
